# revision 27
# baseline (speedup 1.0000x reference)
"""Trainium2 Bass kernel for nn_Patchifier (grid-cell argmax + clamped top-k).

Computation per (b, n) map of shape [480, 640] (128 maps total):
  - split into 10x10 grid of 48x64 cells; per-cell argmax (first occurrence,
    row-major within the cell)
  - clamp argmax coords to [1, 478] x [1, 638]; re-read score at the clamped
    coordinate
  - top-80 of the 100 candidate scores per map, sorted descending with
    jax.lax.top_k tie order (lower candidate index first)
Returns (x_coords int32 [4,32,80], y_coords int32, top_scores f32).

Distribution: 128 maps sharded 16-per-core across 8 NeuronCores (pure data
parallel, no collectives).

Per-core dataflow (maps processed in groups of 4):
  1. HBM -> SBUF raw tile [120, 2560] (partition = 4 consecutive rows);
     DVE reduces to per-ROW-PAIR / per-grid-col maxes [120, (gc, pair)]
     (pairs = adjacent rows (2t, 2t+1), so pair order == row order)
  2. PE-transpose to [20, 120]; DRAM relay rearranges to cell-major
     [100, 24] (one relay-out DMA per map, one readback per group)
  3. cell max + first attaining PAIR p* (value-iota min trick); two
     indirect-DMA gathers per map fetch rows 2p* and 2p*+1 of every cell
     (the row-clamped coordinate is always the pair sibling: row 0 clamps
     to 1, row 479 to 478, both inside pair 0 / pair 23)
  4. one 128-wide value-iota pass over [row0|row1] chunks gives the exact
     row-major argmax (h*, c*); coords clamped; score extracted at the
     clamped (row, col) via a 128-wide one-hot + accumulate
  5. exact sort ranks per map: rank_i = #(v_j > v_i) (+ #(j<i, v_j == v_i)
     tie term, emitted only when the input actually contains tied
     candidate scores); one-hot of rank < 80; per-map PE matmul gathers
     (score, x, y) into sorted order
  6. final PE transpose -> [48, 80]; 3 output DMAs
"""

import sys

if "/opt/trn_rl_repo" not in sys.path:
    sys.path.insert(0, "/opt/trn_rl_repo")

import numpy as np

import concourse.bacc as bacc
import concourse.bass as bass
import concourse.mybir as mybir
from concourse.bass import IndirectOffsetOnAxis
from concourse.tile import TileContext
from concourse.bass_utils import run_bass_kernel_spmd

F32 = mybir.dt.float32
I32 = mybir.dt.int32

N_CORES = 8
MAPS = 16          # maps per core
GROUP = 4          # maps per group
H, W = 480, 640
G = 10             # grid
CH, CW = 48, 64    # cell size
NPAIR = 24         # row pairs per cell
NCAND = 100        # candidate cells per map
TOPN = 80
AX = mybir.AluOpType

# packed constant table: column ranges in the [128, C_W] const tile
C_IDENT = (0, 128)
C_ROWB = (128, 129)    # 48*g + 2000  (2*pstar' + this = argmax pair row0)
C_MAPGC = (129, 145)   # m*4800 + gc  (chunk index offset per map)
C_CGC64 = (145, 146)   # 64*gc
C_CGC64B = (146, 147)  # 64*gc + 4096 (un-biases the 128-wide viota)
C_VIOTA24 = (147, 171)  # relay f=12*r2+s -> pair 2s+r2, biased -1000
C_VIOTA128 = (171, 299)  # h*64+c - 4096
C_IOTA128 = (299, 427)  # h*64+c (for the extract one-hot)
C_IOTA80 = (427, 507)
C_TRI = (507, 607)     # [j < i] (only used when ties exist)
C_ONES = (607, 707)   # all-ones block (ones-row matmul broadcast)
C_W = 707


def _consts():
    c = np.zeros((128, C_W), dtype=np.float32)
    c[:128, C_IDENT[0]:C_IDENT[1]] = np.eye(128, dtype=np.float32)

    cell = np.arange(NCAND)
    g = cell // G
    gc = cell % G
    c[:NCAND, C_ROWB[0]] = (g * CH + 2000).astype(np.float32)
    c[:NCAND, C_MAPGC[0]:C_MAPGC[1]] = (
        np.arange(MAPS)[None, :] * (H * W // CW) + gc[:, None])
    c[:NCAND, C_CGC64[0]] = (gc * CW).astype(np.float32)
    c[:NCAND, C_CGC64B[0]] = (gc * CW + 4096).astype(np.float32)
    f = np.arange(NPAIR)
    c[:NCAND, C_VIOTA24[0]:C_VIOTA24[1]] = (2 * (f % 12) + f // 12 - 1000)[None, :]
    q = np.arange(2 * CW)
    c[:NCAND, C_VIOTA128[0]:C_VIOTA128[1]] = (q - 4096)[None, :]
    c[:NCAND, C_IOTA128[0]:C_IOTA128[1]] = q[None, :]
    c[:NCAND, C_IOTA80[0]:C_IOTA80[1]] = np.arange(TOPN)[None, :]
    i = np.arange(NCAND)
    c[:NCAND, C_TRI[0]:C_TRI[1]] = (i[None, :] < i[:, None]).astype(np.float32)
    c[:, C_ONES[0]:C_ONES[1]] = 1.0
    return {"ctab": c}


def build_nc(handle_ties: bool):
    nc = bacc.Bacc()

    score = nc.dram_tensor("score", [MAPS, H, W], F32, kind="ExternalInput")
    ctab_d = nc.dram_tensor("ctab", [128, C_W], F32, kind="ExternalInput")

    x_out = nc.dram_tensor("x_out", [MAPS, TOPN], I32, kind="ExternalOutput")
    y_out = nc.dram_tensor("y_out", [MAPS, TOPN], I32, kind="ExternalOutput")
    s_out = nc.dram_tensor("s_out", [MAPS, TOPN], F32, kind="ExternalOutput")

    relay = nc.dram_tensor("relay", [MAPS, NCAND, NPAIR], F32)

    # flat chunk view of the input: chunk i = 64 contiguous floats
    score_chunks = score[:, :, :].rearrange("m r (gc c) -> (m r gc) c", c=CW)

    with TileContext(nc) as tc:
        with (
            tc.tile_pool(name="raw", bufs=4) as rawp,
            tc.tile_pool(name="rt", bufs=2) as rtp,
            tc.tile_pool(name="scr", bufs=2) as jp,
            tc.tile_pool(name="small", bufs=1) as sp,
            tc.tile_pool(name="psum", bufs=2, space="PSUM") as pp,
            tc.tile_pool(name="psumv", bufs=2, space="PSUM") as ppv,
            tc.tile_pool(name="psum1", bufs=1, space="PSUM") as pp1,
        ):
            cst = sp.tile([128, C_W], F32, tag="ctab")
            nc.sync.dma_start(out=cst[:, :], in_=ctab_d[:, :])

            def cc(rng):
                return cst[0:NCAND, rng[0]:rng[1]]

            ident = cst[:, C_IDENT[0]:C_IDENT[1]]

            # persistent per-candidate state, free dim = map
            cells24 = sp.tile([NCAND, MAPS * NPAIR], F32, tag="cells24")
            cellmax = sp.tile([NCAND, MAPS], F32, tag="cellmax")
            pstar = sp.tile([NCAND, MAPS], F32, tag="pstar")
            rowA0 = sp.tile([NCAND, MAPS], F32, tag="rowA0")
            chA0f = sp.tile([NCAND, MAPS], F32, tag="chA0f")
            chA1f = sp.tile([NCAND, MAPS], F32, tag="chA1f")
            chA0i = sp.tile([NCAND, MAPS], I32, tag="chA0i")
            chA1i = sp.tile([NCAND, MAPS], I32, tag="chA1i")
            chunk2 = sp.tile([NCAND, MAPS * 2 * CW], F32, tag="chunk2")
            qstar = sp.tile([NCAND, MAPS], F32, tag="qstar")
            hstar = sp.tile([NCAND, MAPS], F32, tag="hstar")
            e0 = sp.tile([NCAND, MAPS], F32, tag="e0")
            uu = sp.tile([NCAND, MAPS], F32, tag="uu")
            ccl = sp.tile([NCAND, MAPS], F32, tag="ccl")
            idx = sp.tile([NCAND, MAPS], F32, tag="idx")
            rank = sp.tile([NCAND, MAPS], F32, tag="rank")
            # S3v: per map (score, x, y) in columns 3m, 3m+1, 3m+2
            S3v = sp.tile([NCAND, 3 * MAPS], F32, tag="S3v")
            S3r = S3v[:, :].rearrange("p (mm e) -> p mm e", e=3)
            xyS_ps = pp1.tile([TOPN, 3 * MAPS], F32, tag="xyS_ps")

            n_groups = MAPS // GROUP
            for grp in range(n_groups):
                m0 = grp * GROUP
                rtg = rtp.tile([20, GROUP * 120], F32, tag="rtg")
                for k in range(GROUP):
                    m = m0 + k
                    raw = rawp.tile([120, 2560], F32, tag="raw")
                    nc.sync.dma_start(
                        out=raw[:, :],
                        in_=score[m:m + 1, :, :].rearrange(
                            "mm (p rr) c -> (mm p) (rr c)", rr=4
                        ),
                    )
                    # per-(row-pair, grid-col) max: pairs (4t, 4t+1), (4t+2, 4t+3)
                    rmx = jp.tile([120, 20], F32, tag="rmx")
                    rmxv = rmx[:, :].rearrange("p (gc r2) -> p r2 gc", r2=2)
                    rawv = raw[:, :].rearrange(
                        "p (r2 rp gc c) -> p r2 gc rp c", r2=2, rp=2, gc=G)
                    for r2 in range(2):
                        nc.vector.tensor_reduce(
                            out=rmxv[:, r2:r2 + 1, :].squeeze(1),
                            in_=rawv[:, r2:r2 + 1].squeeze(1),
                            axis=mybir.AxisListType.XY,
                            op=AX.max,
                        )
                    ps_rt = pp.tile([20, 120], F32, tag="ps_rt")
                    nc.tensor.transpose(
                        out=ps_rt[:, :], in_=rmx[:, :],
                        identity=ident[:120, :120],
                    )
                    nc.scalar.copy(
                        out=rtg[:, k * 120:(k + 1) * 120], in_=ps_rt[:, :]
                    )
                    # DRAM relay: addr = 24*cell + 12*r2 + s  (cell = 10g+gc)
                    nc.scalar.dma_start(
                        out=relay[m:m + 1, :, :].rearrange(
                            "mm (g2 gc) (r2 s) -> mm gc r2 g2 s", gc=G, s=12
                        ),
                        in_=rtg[:, k * 120:(k + 1) * 120].rearrange(
                            "q (g2 s) -> q g2 s", s=12
                        ),
                    )

                csl = slice(m0 * NPAIR, (m0 + GROUP) * NPAIR)
                nc.scalar.dma_start(
                    out=cells24[:, csl],
                    in_=relay[m0:m0 + GROUP, :, :].rearrange(
                        "mm cell f -> cell mm f"
                    ),
                )

                sl = slice(m0, m0 + GROUP)
                c24 = cells24[:, csl].rearrange("p (mm f) -> p mm f", f=NPAIR)
                # ---- cell max + first attaining pair ----
                nc.vector.tensor_reduce(
                    out=cellmax[:, sl], in_=c24,
                    axis=mybir.AxisListType.X, op=AX.max,
                )
                j24 = jp.tile([NCAND, GROUP * NPAIR], F32, tag="j24")
                j24v = j24[:, :].rearrange("p (mm f) -> p mm f", f=NPAIR)
                nc.vector.tensor_tensor(
                    out=j24v, in0=c24,
                    in1=cellmax[:, sl].unsqueeze(2).broadcast_to(
                        [NCAND, GROUP, NPAIR]),
                    op=AX.is_ge)
                nc.vector.tensor_tensor(
                    out=j24v, in0=j24v,
                    in1=cc(C_VIOTA24).unsqueeze(1).broadcast_to(
                        [NCAND, GROUP, NPAIR]),
                    op=AX.mult)
                nc.vector.tensor_reduce(
                    out=pstar[:, sl], in_=j24v,
                    axis=mybir.AxisListType.X, op=AX.min,
                )

                # ---- gather indices: rows 2*pstar and +1 of every cell ----
                # pstar is biased -1000; C_ROWB carries 48g + 2*1000
                nc.vector.tensor_scalar(
                    out=rowA0[:, sl], in0=pstar[:, sl],
                    scalar1=2.0, scalar2=cc(C_ROWB),
                    op0=AX.mult, op1=AX.add)
                nc.vector.scalar_tensor_tensor(
                    out=chA0f[:, sl], in0=rowA0[:, sl], scalar=float(G),
                    in1=cst[0:NCAND, C_MAPGC[0] + m0:C_MAPGC[0] + m0 + GROUP],
                    op0=AX.mult, op1=AX.add)
                nc.vector.tensor_scalar_add(chA1f[:, sl], chA0f[:, sl], 10.0)
                nc.vector.tensor_copy(chA0i[:, sl], chA0f[:, sl])
                nc.vector.tensor_copy(chA1i[:, sl], chA1f[:, sl])

                ck2 = chunk2[:, :].rearrange(
                    "p (mm h c) -> p mm h c", h=2, c=CW)
                for k in range(GROUP):
                    m = m0 + k
                    nc.gpsimd.indirect_dma_start(
                        out=ck2[:, m:m + 1, 0:1].squeeze(1).squeeze(1),
                        out_offset=None,
                        in_=score_chunks,
                        in_offset=IndirectOffsetOnAxis(
                            ap=chA0i[:, m:m + 1], axis=0),
                    )
                    nc.gpsimd.indirect_dma_start(
                        out=ck2[:, m:m + 1, 1:2].squeeze(1).squeeze(1),
                        out_offset=None,
                        in_=score_chunks,
                        in_offset=IndirectOffsetOnAxis(
                            ap=chA1i[:, m:m + 1], axis=0),
                    )

                # ---- 128-wide row-major argmax over the gathered pair ----
                ck2g = chunk2[:, m0 * 2 * CW:(m0 + GROUP) * 2 * CW].rearrange(
                    "p (mm f) -> p mm f", f=2 * CW)
                j128 = jp.tile([NCAND, GROUP * 2 * CW], F32, tag="j128")
                j128v = j128[:, :].rearrange("p (mm f) -> p mm f", f=2 * CW)
                nc.vector.tensor_tensor(
                    out=j128v, in0=ck2g,
                    in1=cellmax[:, sl].unsqueeze(2).broadcast_to(
                        [NCAND, GROUP, 2 * CW]),
                    op=AX.is_ge)
                nc.vector.tensor_tensor(
                    out=j128v, in0=j128v,
                    in1=cc(C_VIOTA128).unsqueeze(1).broadcast_to(
                        [NCAND, GROUP, 2 * CW]),
                    op=AX.mult)
                nc.vector.tensor_reduce(
                    out=qstar[:, sl], in_=j128v,
                    axis=mybir.AxisListType.X, op=AX.min,
                )
                # qstar = h*64 + c - 4096; split h (row-in-pair) and column
                nc.vector.tensor_scalar(
                    out=hstar[:, sl], in0=qstar[:, sl],
                    scalar1=float(CW - 4096), scalar2=None, op0=AX.is_ge)
                # x coordinate: clamp(rowA0 + h, 1, 478)
                xsl = S3r[:, m0:m0 + GROUP, 1:2].squeeze(2)
                nc.vector.tensor_tensor(
                    out=e0[:, sl], in0=rowA0[:, sl], in1=hstar[:, sl],
                    op=AX.add)
                nc.vector.tensor_scalar_max(e0[:, sl], e0[:, sl], 1.0)
                nc.vector.tensor_scalar_min(xsl, e0[:, sl], float(H - 2))
                # row actually read = the clamped row; it is always within
                # the gathered pair: e0 = which half
                nc.vector.tensor_tensor(
                    out=e0[:, sl], in0=xsl, in1=rowA0[:, sl], op=AX.subtract)
                # y coordinate: c = qstar - 64*h + 4096; clamp(64gc + c)
                nc.vector.scalar_tensor_tensor(
                    out=uu[:, sl], in0=hstar[:, sl], scalar=-float(CW),
                    in1=qstar[:, sl], op0=AX.mult, op1=AX.add)
                ysl = S3r[:, m0:m0 + GROUP, 2:3].squeeze(2)
                nc.vector.tensor_scalar_add(uu[:, sl], uu[:, sl], cc(C_CGC64B))
                nc.vector.tensor_scalar_max(uu[:, sl], uu[:, sl], 1.0)
                nc.vector.tensor_scalar_min(ysl, uu[:, sl], float(W - 2))
                nc.vector.tensor_scalar(
                    out=ccl[:, sl], in0=ysl,
                    scalar1=cc(C_CGC64), scalar2=None, op0=AX.subtract)
                # extract score at (clamped row, clamped col): one-hot over
                # the 128-wide pair chunk at position e0*64 + ccl
                nc.vector.scalar_tensor_tensor(
                    out=idx[:, sl], in0=e0[:, sl], scalar=float(CW),
                    in1=ccl[:, sl], op0=AX.mult, op1=AX.add)
                for k in range(GROUP):
                    m = m0 + k
                    junk = jp.tile([NCAND, 2 * CW], F32, tag="junk")
                    nc.vector.scalar_tensor_tensor(
                        out=junk[:, :], in0=cc(C_IOTA128),
                        scalar=idx[:, m:m + 1],
                        in1=chunk2[:, m * 2 * CW:(m + 1) * 2 * CW],
                        op0=AX.is_equal, op1=AX.mult,
                        accum_out=S3r[:, m:m + 1, 0:1].squeeze(1),
                    )

                # ---- exact sort ranks + one-hot + gather matmul ----
                vflat = jp.tile([1, GROUP * NCAND], F32, tag="vflat")
                nc.scalar.dma_start(
                    out=vflat[0:1, :].rearrange(
                        "o (j mm) -> o j mm", mm=GROUP),
                    in_=S3r[:, m0:m0 + GROUP, 0:1].squeeze(2),
                )
                vrep_ps = ppv.tile([NCAND, GROUP * NCAND], F32, tag="vrep")
                nc.tensor.matmul(
                    out=vrep_ps[:, :],
                    lhsT=cst[0:1, C_ONES[0]:C_ONES[1]],
                    rhs=vflat[0:1, :],
                    start=True, stop=True)
                vj = vrep_ps[:, :].rearrange("p (j mm) -> p mm j", mm=GROUP)
                vi = S3r[:, m0:m0 + GROUP, 0:1].broadcast_to(
                    [NCAND, GROUP, NCAND])
                jA = jp.tile([NCAND, GROUP * NCAND], F32, tag="jA")
                jAv = jA[:, :].rearrange("p (mm f) -> p mm f", f=NCAND)
                nc.vector.tensor_tensor(out=jAv, in0=vj, in1=vi, op=AX.is_gt)
                if handle_ties:
                    jB = jp.tile([NCAND, GROUP * NCAND], F32, tag="jB")
                    jBv = jB[:, :].rearrange("p (mm f) -> p mm f", f=NCAND)
                    nc.vector.tensor_tensor(
                        out=jBv, in0=vj, in1=vi, op=AX.is_equal)
                    nc.vector.tensor_tensor(
                        out=jBv, in0=jBv,
                        in1=cc(C_TRI).unsqueeze(1).broadcast_to(
                            [NCAND, GROUP, NCAND]),
                        op=AX.mult)
                    nc.vector.tensor_tensor(
                        out=jAv, in0=jAv, in1=jBv, op=AX.add)
                nc.vector.tensor_reduce(
                    out=rank[:, sl], in_=jAv,
                    axis=mybir.AxisListType.X, op=AX.add,
                )
                OHt = jp.tile([NCAND, GROUP * TOPN], F32, tag="OHt")
                nc.vector.tensor_tensor(
                    out=OHt[:, :].rearrange("p (mm f) -> p mm f", f=TOPN),
                    in0=rank[:, sl].unsqueeze(2).broadcast_to(
                        [NCAND, GROUP, TOPN]),
                    in1=cc(C_IOTA80).unsqueeze(1).broadcast_to(
                        [NCAND, GROUP, TOPN]),
                    op=AX.is_equal,
                )
                for k in range(GROUP):
                    m = m0 + k
                    nc.tensor.matmul(
                        out=xyS_ps[:, 3 * m:3 * m + 3],
                        lhsT=OHt[:, k * TOPN:(k + 1) * TOPN],
                        rhs=S3v[:, 3 * m:3 * m + 3],
                        start=True, stop=True)

            # ---- tail: transpose sorted (s, x, y) and write outputs ----
            # evacuate reordered (e, m)-major so transposed rows land
            # s:0-15, x:16-31, y:32-47
            xyS = sp.tile([TOPN, 3 * MAPS], F32, tag="xyS")
            nc.vector.tensor_copy(
                xyS[:, :].rearrange("p (e mm) -> p e mm", mm=MAPS),
                xyS_ps[:, :].rearrange("p (mm e) -> p e mm", e=3))
            psT = pp1.tile([3 * MAPS, TOPN], F32, tag="psT")
            nc.tensor.transpose(
                out=psT[:, :], in_=xyS[:, :],
                identity=ident[:TOPN, :TOPN],
            )
            xyi = sp.tile([3 * MAPS, TOPN], I32, tag="xyi")
            nc.vector.tensor_copy(xyi[:, :], psT[:, :])
            souts = sp.tile([3 * MAPS, TOPN], F32, tag="souts")
            nc.scalar.copy(out=souts[:, :], in_=psT[:, :])
            nc.sync.dma_start(out=s_out[:, :], in_=souts[0:MAPS, :])
            nc.sync.dma_start(out=x_out[:, :], in_=xyi[MAPS:2 * MAPS, :])
            nc.sync.dma_start(out=y_out[:, :], in_=xyi[2 * MAPS:3 * MAPS, :])

    nc.compile()
    return nc


_NC = {}


def _get_nc(handle_ties: bool):
    if handle_ties not in _NC:
        _NC[handle_ties] = build_nc(handle_ties)
    return _NC[handle_ties]


def _has_ties(flat: np.ndarray) -> bool:
    """True if any map's 100 candidate scores contain duplicates."""
    n = flat.shape[0]
    cells = (flat.reshape(n, G, CH, G, CW).transpose(0, 1, 3, 2, 4)
             .reshape(n, G * G, CH * CW))
    fi = cells.argmax(axis=-1)
    r = fi // CW
    c = fi % CW
    g = np.arange(G * G) // G
    gc = np.arange(G * G) % G
    gr = np.clip(g[None, :] * CH + r, 1, H - 2)
    gcl = np.clip(gc[None, :] * CW + c, 1, W - 2)
    lin = gr * W + gcl
    cand = np.take_along_axis(flat.reshape(n, H * W), lin, axis=-1)
    sorted_cand = np.sort(cand, axis=-1)
    return bool((sorted_cand[:, 1:] == sorted_cand[:, :-1]).any())


def kernel(score_maps: np.ndarray, top_n=80, _trace=False):
    score_maps = np.ascontiguousarray(np.asarray(score_maps), dtype=np.float32)
    assert score_maps.shape == (4, 32, H, W), score_maps.shape
    assert int(top_n) == TOPN

    flat = score_maps.reshape(4 * 32, H, W)
    nc = _get_nc(_has_ties(flat))
    consts = _consts()
    in_maps = []
    for c in range(N_CORES):
        m = {"score": np.ascontiguousarray(flat[c * MAPS:(c + 1) * MAPS])}
        m.update(consts)
        in_maps.append(m)

    res = run_bass_kernel_spmd(nc, in_maps, list(range(N_CORES)), trace=_trace)
    x = np.concatenate([res.results[c]["x_out"] for c in range(N_CORES)])
    y = np.concatenate([res.results[c]["y_out"] for c in range(N_CORES)])
    s = np.concatenate([res.results[c]["s_out"] for c in range(N_CORES)])
    return (
        x.reshape(4, 32, TOPN).astype(np.int32),
        y.reshape(4, 32, TOPN).astype(np.int32),
        s.reshape(4, 32, TOPN).astype(np.float32),
    )


# revision 42
# speedup vs baseline: 1.1540x; 1.1540x over previous
"""Trainium2 Bass kernel for nn_Patchifier (grid-cell argmax + clamped top-k).

Computation per (b, n) map of shape [480, 640] (128 maps total):
  - split into 10x10 grid of 48x64 cells; per-cell argmax (first occurrence,
    row-major within the cell)
  - clamp argmax coords to [1, 478] x [1, 638]; re-read score at the clamped
    coordinate
  - top-80 of the 100 candidate scores per map, sorted descending with
    jax.lax.top_k tie order (lower candidate index first)
Returns (x_coords int32 [4,32,80], y_coords int32, top_scores f32).

Distribution: 128 maps sharded 16-per-core across 8 NeuronCores (pure data
parallel, no collectives).

Per-core dataflow (maps processed in groups of 4):
  1. HBM -> SBUF raw tile [120, 2560] (partition = 4 consecutive rows);
     DVE reduces to per-ROW-PAIR / per-grid-col maxes [120, (gc, pair)]
     (pairs = adjacent rows (2t, 2t+1), so pair order == row order)
  2. PE-transpose to [20, 120]; DRAM relay rearranges to cell-major
     [100, 24] (one relay-out DMA per map, one readback per group)
  3. cell max + first attaining PAIR p* (value-iota min trick); two
     indirect-DMA gathers per map fetch rows 2p* and 2p*+1 of every cell
     (the row-clamped coordinate is always the pair sibling: row 0 clamps
     to 1, row 479 to 478, both inside pair 0 / pair 23)
  4. one 128-wide value-iota pass over [row0|row1] chunks gives the exact
     row-major argmax (h*, c*); coords clamped; score extracted at the
     clamped (row, col) via a 128-wide one-hot + accumulate
  5. exact sort ranks per map: rank_i = #(v_j > v_i) (+ #(j<i, v_j == v_i)
     tie term, emitted only when the input actually contains tied
     candidate scores); one-hot of rank < 80; per-map PE matmul gathers
     (score, x, y) into sorted order
  6. final PE transpose -> [48, 80]; 3 output DMAs
"""

import sys

if "/opt/trn_rl_repo" not in sys.path:
    sys.path.insert(0, "/opt/trn_rl_repo")

import numpy as np

import concourse.bacc as bacc
import concourse.bass as bass
import concourse.mybir as mybir
from concourse.bass import IndirectOffsetOnAxis
from concourse.tile import TileContext
from concourse.bass_utils import run_bass_kernel_spmd

F32 = mybir.dt.float32
I32 = mybir.dt.int32

N_CORES = 8
MAPS = 16          # maps per core
GROUP = 4          # maps per group
H, W = 480, 640
G = 10             # grid
CH, CW = 48, 64    # cell size
NPAIR = 24         # row pairs per cell
NCAND = 100        # candidate cells per map
TOPN = 80
AX = mybir.AluOpType

# packed constant table: column ranges in the [128, C_W] const tile
C_IDENT = (0, 128)
C_ROWB = (128, 129)    # 48*g + 2000  (2*pstar' + this = argmax pair row0)
C_MAPGC = (129, 145)   # m*4800 + gc  (chunk index offset per map)
C_CGC64 = (145, 146)   # 64*gc
C_CGC64B = (146, 147)  # 64*gc + 4096 (un-biases the 128-wide viota)
C_VIOTA24 = (147, 171)  # relay f=12*r2+s -> pair 2s+r2, biased -1000
C_VIOTA128 = (171, 299)  # h*64+c - 4096
C_IOTA128 = (299, 427)  # h*64+c (for the extract one-hot)
C_IOTA80 = (427, 507)
C_TRI = (507, 607)     # [j < i] (only used when ties exist)
C_ONES = (607, 707)   # all-ones block (ones-row matmul broadcast)
C_W = 707


def _consts():
    c = np.zeros((128, C_W), dtype=np.float32)
    c[:128, C_IDENT[0]:C_IDENT[1]] = np.eye(128, dtype=np.float32)

    cell = np.arange(NCAND)
    g = cell // G
    gc = cell % G
    c[:NCAND, C_ROWB[0]] = (g * CH + 2000).astype(np.float32)
    c[:NCAND, C_MAPGC[0]:C_MAPGC[1]] = (
        np.arange(MAPS)[None, :] * (H * W // CW) + gc[:, None])
    c[:NCAND, C_CGC64[0]] = (gc * CW).astype(np.float32)
    c[:NCAND, C_CGC64B[0]] = (gc * CW + 4096).astype(np.float32)
    f = np.arange(NPAIR)
    c[:NCAND, C_VIOTA24[0]:C_VIOTA24[1]] = (2 * (f % 12) + f // 12 - 1000)[None, :]
    q = np.arange(2 * CW)
    c[:NCAND, C_VIOTA128[0]:C_VIOTA128[1]] = (q - 4096)[None, :]
    c[:NCAND, C_IOTA128[0]:C_IOTA128[1]] = q[None, :]
    c[:NCAND, C_IOTA80[0]:C_IOTA80[1]] = np.arange(TOPN)[None, :]
    i = np.arange(NCAND)
    c[:NCAND, C_TRI[0]:C_TRI[1]] = (i[None, :] < i[:, None]).astype(np.float32)
    c[:, C_ONES[0]:C_ONES[1]] = 1.0
    return {"ctab": c}


def build_nc(handle_ties: bool):
    nc = bacc.Bacc()

    score = nc.dram_tensor("score", [MAPS, H, W], F32, kind="ExternalInput")
    ctab_d = nc.dram_tensor("ctab", [128, C_W], F32, kind="ExternalInput")

    x_out = nc.dram_tensor("x_out", [MAPS, TOPN], I32, kind="ExternalOutput")
    y_out = nc.dram_tensor("y_out", [MAPS, TOPN], I32, kind="ExternalOutput")
    s_out = nc.dram_tensor("s_out", [MAPS, TOPN], F32, kind="ExternalOutput")

    relay = nc.dram_tensor("relay", [MAPS, NCAND, NPAIR], F32)

    # flat chunk view of the input: chunk i = 64 contiguous floats
    score_chunks = score[:, :, :].rearrange("m r (gc c) -> (m r gc) c", c=CW)

    with TileContext(nc) as tc:
        with (
            tc.tile_pool(name="raw", bufs=3) as rawp,
            tc.tile_pool(name="rt", bufs=3) as rtp,
            tc.tile_pool(name="scr", bufs=2) as jp,
            tc.tile_pool(name="small", bufs=1) as sp,
            tc.tile_pool(name="psum", bufs=3, space="PSUM") as pp,
            tc.tile_pool(name="psumv", bufs=1, space="PSUM") as ppv,
            tc.tile_pool(name="psumt", bufs=1, space="PSUM") as ppt,
            tc.tile_pool(name="psum1", bufs=1, space="PSUM") as pp1,
        ):
            cst = sp.tile([128, C_W], F32, tag="ctab")
            nc.sync.dma_start(out=cst[:, :], in_=ctab_d[:, :])

            def cc(rng):
                return cst[0:NCAND, rng[0]:rng[1]]

            ident = cst[:, C_IDENT[0]:C_IDENT[1]]

            # persistent per-candidate state, free dim = map
            cells24 = sp.tile([NCAND, MAPS * NPAIR], F32, tag="cells24")
            cellmax = sp.tile([NCAND, MAPS], F32, tag="cellmax")
            pstar = sp.tile([NCAND, MAPS], F32, tag="pstar")
            rowA0 = sp.tile([NCAND, MAPS], F32, tag="rowA0")
            chA0f = sp.tile([NCAND, MAPS], F32, tag="chA0f")
            chA1f = sp.tile([NCAND, MAPS], F32, tag="chA1f")
            chA0i = sp.tile([NCAND, MAPS], I32, tag="chA0i")
            chA1i = sp.tile([NCAND, MAPS], I32, tag="chA1i")
            chunk2 = sp.tile([NCAND, MAPS * 2 * CW], F32, tag="chunk2")
            qstar = sp.tile([NCAND, MAPS], F32, tag="qstar")
            hstar = sp.tile([NCAND, MAPS], F32, tag="hstar")
            e0 = sp.tile([NCAND, MAPS], F32, tag="e0")
            uu = sp.tile([NCAND, MAPS], F32, tag="uu")
            ccl = sp.tile([NCAND, MAPS], F32, tag="ccl")
            idx = sp.tile([NCAND, MAPS], F32, tag="idx")
            rank = sp.tile([NCAND, MAPS], F32, tag="rank")
            # S3v: per map (score, x, y) in columns 3m, 3m+1, 3m+2
            S3v = sp.tile([NCAND, 3 * MAPS], F32, tag="S3v")
            S3r = S3v[:, :].rearrange("p (mm e) -> p mm e", e=3)
            xyS_ps = pp1.tile([TOPN, 3 * MAPS], F32, tag="xyS_ps")

            # software-pipelined emission (priority = emission order):
            # loads/reduces for group g+1 are emitted BEFORE the phase chain
            # of group g, so the scheduler overlaps the phase chains with the
            # streaming input DMAs + reduces
            def emit_load(m):
                k = m % GROUP
                raw = rawp.tile([120, 2560], F32, tag="raw")
                # two half-map DMAs: smaller FIFO quanta on the DMA engines
                # let the phase-critical small DMAs slot in sooner
                iv = score[m:m + 1, :, :].rearrange(
                    "mm (hh p rr) c -> hh (mm p) (rr c)", hh=2, rr=4)
                nc.sync.dma_start(out=raw[0:60, :], in_=iv[0:1].squeeze(0))
                nc.sync.dma_start(out=raw[60:120, :], in_=iv[1:2].squeeze(0))
                # per-(row-pair, grid-col) max: pairs (4t, 4t+1), (4t+2, 4t+3)
                rmx = jp.tile([120, 20], F32, tag="rmx")
                rmxv = rmx[:, :].rearrange("p (gc r2) -> p r2 gc", r2=2)
                rawv = raw[:, :].rearrange(
                    "p (r2 rp gc c) -> p r2 gc rp c", r2=2, rp=2, gc=G)
                for r2 in range(2):
                    nc.vector.tensor_reduce(
                        out=rmxv[:, r2:r2 + 1, :].squeeze(1),
                        in_=rawv[:, r2:r2 + 1].squeeze(1),
                        axis=mybir.AxisListType.XY,
                        op=AX.max,
                    )
                ps_rt = pp.tile([20, 120], F32, tag="ps_rt")
                nc.tensor.transpose(
                    out=ps_rt[:, :], in_=rmx[:, :],
                    identity=ident[:120, :120],
                )
                rt = rtp.tile([20, 120], F32, tag="rt")
                nc.scalar.copy(out=rt[:, :], in_=ps_rt[:, :])
                # DRAM relay: addr = 24*cell + 12*r2 + s  (cell = 10g+gc)
                nc.scalar.dma_start(
                    out=relay[m:m + 1, :, :].rearrange(
                        "mm (g2 gc) (r2 s) -> mm gc r2 g2 s", gc=G, s=12
                    ),
                    in_=rt[:, :].rearrange(
                        "q (g2 s) -> q g2 s", s=12
                    ),
                )


            def emit_A(m0, gsz):
                # readback + cell max + attaining pair + gather dispatch
                csl = slice(m0 * NPAIR, (m0 + gsz) * NPAIR)
                nc.scalar.dma_start(
                    out=cells24[:, csl],
                    in_=relay[m0:m0 + gsz, :, :].rearrange(
                        "mm cell f -> cell mm f"
                    ),
                )

                sl = slice(m0, m0 + gsz)
                c24 = cells24[:, csl].rearrange("p (mm f) -> p mm f", f=NPAIR)
                # ---- cell max + first attaining pair ----
                nc.vector.tensor_reduce(
                    out=cellmax[:, sl], in_=c24,
                    axis=mybir.AxisListType.X, op=AX.max,
                )
                j24 = jp.tile([NCAND, GROUP * NPAIR], F32, tag="j24")
                j24v = j24[:, :GROUP * NPAIR].rearrange(
                    "p (mm f) -> p mm f", f=NPAIR)[:, :gsz]
                nc.vector.tensor_tensor(
                    out=j24v, in0=c24,
                    in1=cellmax[:, sl].unsqueeze(2).broadcast_to(
                        [NCAND, gsz, NPAIR]),
                    op=AX.is_ge)
                nc.vector.tensor_tensor(
                    out=j24v, in0=j24v,
                    in1=cc(C_VIOTA24).unsqueeze(1).broadcast_to(
                        [NCAND, gsz, NPAIR]),
                    op=AX.mult)
                nc.vector.tensor_reduce(
                    out=pstar[:, sl], in_=j24v,
                    axis=mybir.AxisListType.X, op=AX.min,
                )

                # ---- gather indices: rows 2*pstar and +1 of every cell ----
                # pstar is biased -1000; C_ROWB carries 48g + 2*1000
                nc.vector.tensor_scalar(
                    out=rowA0[:, sl], in0=pstar[:, sl],
                    scalar1=2.0, scalar2=cc(C_ROWB),
                    op0=AX.mult, op1=AX.add)
                nc.vector.scalar_tensor_tensor(
                    out=chA0f[:, sl], in0=rowA0[:, sl], scalar=float(G),
                    in1=cst[0:NCAND, C_MAPGC[0] + m0:C_MAPGC[0] + m0 + gsz],
                    op0=AX.mult, op1=AX.add)
                nc.vector.tensor_scalar_add(chA1f[:, sl], chA0f[:, sl], 10.0)
                nc.vector.tensor_copy(chA0i[:, sl], chA0f[:, sl])
                nc.vector.tensor_copy(chA1i[:, sl], chA1f[:, sl])

                ck2 = chunk2[:, :].rearrange(
                    "p (mm h c) -> p mm h c", h=2, c=CW)
                for k in range(gsz):
                    m = m0 + k
                    nc.gpsimd.indirect_dma_start(
                        out=ck2[:, m:m + 1, 0:1].squeeze(1).squeeze(1),
                        out_offset=None,
                        in_=score_chunks,
                        in_offset=IndirectOffsetOnAxis(
                            ap=chA0i[:, m:m + 1], axis=0),
                    )
                    nc.gpsimd.indirect_dma_start(
                        out=ck2[:, m:m + 1, 1:2].squeeze(1).squeeze(1),
                        out_offset=None,
                        in_=score_chunks,
                        in_offset=IndirectOffsetOnAxis(
                            ap=chA1i[:, m:m + 1], axis=0),
                    )

            def emit_B(m0, gsz):
                # 128-wide row-major argmax + coords + score extract
                sl = slice(m0, m0 + gsz)
                j128 = jp.tile([NCAND, GROUP * 2 * CW], F32, tag="j128")
                for k in range(gsz):
                    m = m0 + k
                    nc.vector.scalar_tensor_tensor(
                        out=j128[:, k * 2 * CW:(k + 1) * 2 * CW],
                        in0=chunk2[:, m * 2 * CW:(m + 1) * 2 * CW],
                        scalar=cellmax[:, m:m + 1],
                        in1=cc(C_VIOTA128),
                        op0=AX.is_ge, op1=AX.mult)
                nc.vector.tensor_reduce(
                    out=qstar[:, sl],
                    in_=j128[:, :gsz * 2 * CW].rearrange(
                        "p (mm f) -> p mm f", f=2 * CW),
                    axis=mybir.AxisListType.X, op=AX.min,
                )
                # qstar = h*64 + c - 4096; split h (row-in-pair) and column
                nc.vector.tensor_scalar(
                    out=hstar[:, sl], in0=qstar[:, sl],
                    scalar1=float(CW - 4096), scalar2=None, op0=AX.is_ge)
                # x coordinate: clamp(rowA0 + h, 1, 478)
                xsl = S3r[:, m0:m0 + gsz, 1:2].squeeze(2)
                nc.vector.tensor_tensor(
                    out=e0[:, sl], in0=rowA0[:, sl], in1=hstar[:, sl],
                    op=AX.add)
                nc.vector.tensor_scalar_max(e0[:, sl], e0[:, sl], 1.0)
                nc.vector.tensor_scalar_min(xsl, e0[:, sl], float(H - 2))
                # row actually read = the clamped row; it is always within
                # the gathered pair: e0 = which half
                nc.vector.tensor_tensor(
                    out=e0[:, sl], in0=xsl, in1=rowA0[:, sl], op=AX.subtract)
                # y coordinate: c = qstar - 64*h + 4096; clamp(64gc + c)
                nc.vector.scalar_tensor_tensor(
                    out=uu[:, sl], in0=hstar[:, sl], scalar=-float(CW),
                    in1=qstar[:, sl], op0=AX.mult, op1=AX.add)
                ysl = S3r[:, m0:m0 + gsz, 2:3].squeeze(2)
                nc.vector.tensor_scalar_add(uu[:, sl], uu[:, sl], cc(C_CGC64B))
                nc.vector.tensor_scalar_max(uu[:, sl], uu[:, sl], 1.0)
                nc.vector.tensor_scalar_min(ysl, uu[:, sl], float(W - 2))
                nc.vector.tensor_scalar(
                    out=ccl[:, sl], in0=ysl,
                    scalar1=cc(C_CGC64), scalar2=None, op0=AX.subtract)
                # extract score at (clamped row, clamped col): one-hot over
                # the 128-wide pair chunk at position e0*64 + ccl
                nc.vector.scalar_tensor_tensor(
                    out=idx[:, sl], in0=e0[:, sl], scalar=float(CW),
                    in1=ccl[:, sl], op0=AX.mult, op1=AX.add)
                for k in range(gsz):
                    m = m0 + k
                    junk = jp.tile([NCAND, 2 * CW], F32, tag="junk")
                    nc.vector.scalar_tensor_tensor(
                        out=junk[:, :], in0=cc(C_IOTA128),
                        scalar=idx[:, m:m + 1],
                        in1=chunk2[:, m * 2 * CW:(m + 1) * 2 * CW],
                        op0=AX.is_equal, op1=AX.mult,
                        accum_out=S3r[:, m:m + 1, 0:1].squeeze(1),
                    )

            def emit_C(m0, gsz):
                # exact sort ranks + one-hot + gather matmul
                sl = slice(m0, m0 + gsz)
                # broadcast each map's scores to all partitions: column
                # transpose (PE) -> [1, 100] -> ones-row matmul
                vrep_ps = ppv.tile([NCAND, GROUP * NCAND], F32, tag="vrep")
                for k in range(gsz):
                    m = m0 + k
                    vt_ps = ppt.tile([1, NCAND], F32, tag="vt")
                    nc.tensor.transpose(
                        out=vt_ps[:, :],
                        in_=S3r[:, m:m + 1, 0:1].squeeze(2),
                        identity=ident[:NCAND, :NCAND])
                    vt = jp.tile([1, NCAND], F32, tag="vts")
                    nc.scalar.copy(out=vt[:, :], in_=vt_ps[:, :])
                    nc.tensor.matmul(
                        out=vrep_ps[:, k * NCAND:(k + 1) * NCAND],
                        lhsT=cst[0:1, C_ONES[0]:C_ONES[1]],
                        rhs=vt[:, :],
                        start=True, stop=True)
                vj = vrep_ps[:, :gsz * NCAND].rearrange(
                    "p (mm j) -> p mm j", mm=gsz)
                vi = S3r[:, m0:m0 + gsz, 0:1].broadcast_to(
                    [NCAND, gsz, NCAND])
                jA = jp.tile([NCAND, GROUP * NCAND], F32, tag="jA")
                jAv = jA[:, :gsz * NCAND].rearrange(
                    "p (mm f) -> p mm f", f=NCAND)
                nc.vector.tensor_tensor(out=jAv, in0=vj, in1=vi, op=AX.is_gt)
                if handle_ties:
                    jB = jp.tile([NCAND, GROUP * NCAND], F32, tag="jB")
                    jBv = jB[:, :gsz * NCAND].rearrange(
                        "p (mm f) -> p mm f", f=NCAND)
                    nc.vector.tensor_tensor(
                        out=jBv, in0=vj, in1=vi, op=AX.is_equal)
                    nc.vector.tensor_tensor(
                        out=jBv, in0=jBv,
                        in1=cc(C_TRI).unsqueeze(1).broadcast_to(
                            [NCAND, gsz, NCAND]),
                        op=AX.mult)
                    nc.vector.tensor_tensor(
                        out=jAv, in0=jAv, in1=jBv, op=AX.add)
                nc.vector.tensor_reduce(
                    out=rank[:, sl], in_=jAv,
                    axis=mybir.AxisListType.X, op=AX.add,
                )
                OHt = jp.tile([NCAND, GROUP * TOPN], F32, tag="OHt")
                nc.vector.tensor_tensor(
                    out=OHt[:, :gsz * TOPN].rearrange(
                        "p (mm f) -> p mm f", f=TOPN),
                    in0=rank[:, sl].unsqueeze(2).broadcast_to(
                        [NCAND, gsz, TOPN]),
                    in1=cc(C_IOTA80).unsqueeze(1).broadcast_to(
                        [NCAND, gsz, TOPN]),
                    op=AX.is_equal,
                )
                for k in range(gsz):
                    m = m0 + k
                    nc.tensor.matmul(
                        out=xyS_ps[:, 3 * m:3 * m + 3],
                        lhsT=OHt[:, k * TOPN:(k + 1) * TOPN],
                        rhs=S3v[:, 3 * m:3 * m + 3],
                        start=True, stop=True)

            # variable group sizes (small tail groups shorten the
            # critical path after the last input DMA) + time pins that tell
            # the Tile scheduler when each stage's data will actually be
            # ready, so it packs phase work into the input-stream gaps
            GROUPS = [(0, 4), (4, 4), (8, 4), (12, 2), (14, 2)]
            grp_of_map = {}
            for gi, (g0_, gs_) in enumerate(GROUPS):
                for mm_ in range(g0_, g0_ + gs_):
                    grp_of_map[mm_] = gi

            def t_load(m):
                return 2.0 + 3.413 * (m + 1)  # us

            stages = {}
            # A right after its group's relays; B after the gathers have
            # had time; C after B. Tuned empirically against the timeline.
            POS = {
                0: [(3, "A"), (15, "B"), (15, "C")],
                1: [(7, "A"), (15, "B"), (15, "C")],
                2: [(11, "A"), (15, "B"), (15, "C")],
                3: [(13, "A"), (15, "B"), (15, "C")],
                4: [(15, "A"), (15, "B"), (15, "C")],
            }
            for gi, (g0_, gs_) in enumerate(GROUPS):
                last = g0_ + gs_ - 1
                tA = t_load(last) + 4.5
                tB = tA + 1.0 + 2.06 * gs_ + 1.6
                tC = tB + 2.5 + 1.9
                tmap = {"A": tA, "B": tB, "C": tC}
                for pos, st in POS[gi]:
                    stages.setdefault(pos, []).append((st, gi, tmap[st]))

            fns = {"A": emit_A, "B": emit_B, "C": emit_C}
            for m in range(MAPS):
                gi = grp_of_map[m]
                g0_, gs_ = GROUPS[gi]
                # pin each load at its natural stream slot so phase-critical
                # DMAs (readbacks/gathers) win their slots on the DMA engines
                with tc.tile_wait_until((1.5 + 3.413 * m) / 1000.0):
                    emit_load(m)
                if m == MAPS - 1:
                    continue  # trailing stages emitted below in order
                for st, g, tp in sorted(stages.get(m, []),
                                        key=lambda x: x[2]):
                    with tc.tile_wait_until(tp / 1000.0):
                        fns[st](*GROUPS[g])
            trailing = sorted(stages.get(MAPS - 1, []), key=lambda x: x[2])
            for st, g, tp in trailing:
                with tc.tile_wait_until(tp / 1000.0):
                    fns[st](*GROUPS[g])

            # ---- tail: transpose sorted (s, x, y) and write outputs ----
            # evacuate reordered (e, m)-major so transposed rows land
            # s:0-15, x:16-31, y:32-47
            xyS = sp.tile([TOPN, 3 * MAPS], F32, tag="xyS")
            nc.vector.tensor_copy(
                xyS[:, :].rearrange("p (e mm) -> p e mm", mm=MAPS),
                xyS_ps[:, :].rearrange("p (mm e) -> p e mm", e=3))
            psT = pp1.tile([3 * MAPS, TOPN], F32, tag="psT")
            nc.tensor.transpose(
                out=psT[:, :], in_=xyS[:, :],
                identity=ident[:TOPN, :TOPN],
            )
            xyi = sp.tile([3 * MAPS, TOPN], I32, tag="xyi")
            nc.vector.tensor_copy(xyi[:, :], psT[:, :])
            souts = sp.tile([3 * MAPS, TOPN], F32, tag="souts")
            nc.scalar.copy(out=souts[:, :], in_=psT[:, :])
            nc.sync.dma_start(out=s_out[:, :], in_=souts[0:MAPS, :])
            nc.sync.dma_start(out=x_out[:, :], in_=xyi[MAPS:2 * MAPS, :])
            nc.sync.dma_start(out=y_out[:, :], in_=xyi[2 * MAPS:3 * MAPS, :])

    nc.compile()
    return nc


_NC = {}


def _get_nc(handle_ties: bool):
    if handle_ties not in _NC:
        _NC[handle_ties] = build_nc(handle_ties)
    return _NC[handle_ties]


def _has_ties(flat: np.ndarray) -> bool:
    """True if any map's 100 candidate scores contain duplicates."""
    n = flat.shape[0]
    cells = (flat.reshape(n, G, CH, G, CW).transpose(0, 1, 3, 2, 4)
             .reshape(n, G * G, CH * CW))
    fi = cells.argmax(axis=-1)
    r = fi // CW
    c = fi % CW
    g = np.arange(G * G) // G
    gc = np.arange(G * G) % G
    gr = np.clip(g[None, :] * CH + r, 1, H - 2)
    gcl = np.clip(gc[None, :] * CW + c, 1, W - 2)
    lin = gr * W + gcl
    cand = np.take_along_axis(flat.reshape(n, H * W), lin, axis=-1)
    sorted_cand = np.sort(cand, axis=-1)
    return bool((sorted_cand[:, 1:] == sorted_cand[:, :-1]).any())


def kernel(score_maps: np.ndarray, top_n=80, _trace=False):
    score_maps = np.ascontiguousarray(np.asarray(score_maps), dtype=np.float32)
    assert score_maps.shape == (4, 32, H, W), score_maps.shape
    assert int(top_n) == TOPN

    flat = score_maps.reshape(4 * 32, H, W)
    nc = _get_nc(_has_ties(flat))
    consts = _consts()
    in_maps = []
    for c in range(N_CORES):
        m = {"score": np.ascontiguousarray(flat[c * MAPS:(c + 1) * MAPS])}
        m.update(consts)
        in_maps.append(m)

    res = run_bass_kernel_spmd(nc, in_maps, list(range(N_CORES)), trace=_trace)
    x = np.concatenate([res.results[c]["x_out"] for c in range(N_CORES)])
    y = np.concatenate([res.results[c]["y_out"] for c in range(N_CORES)])
    s = np.concatenate([res.results[c]["s_out"] for c in range(N_CORES)])
    return (
        x.reshape(4, 32, TOPN).astype(np.int32),
        y.reshape(4, 32, TOPN).astype(np.int32),
        s.reshape(4, 32, TOPN).astype(np.float32),
    )
